# revision 54
# baseline (speedup 1.0000x reference)
"""GCN (4-layer, PyG-default GCNConv) forward on 8 Trainium2 NeuronCores.

Strategy (node-parallel / graph-parallel):
  - Nodes are partitioned contiguously across the 8 cores (1250 rows each,
    padded to 1280 = 10 blocks of 128).
  - Per layer: each core computes its row-slice of G = H @ W as a tiled PE
    GEMM (bf16 in / fp32 accumulate). G is quantized to fp8e4m3 and
    AllGathered in TWO halves (source blocks 0-4 -> "lo", 5-9 -> "hi"): the
    lo AllGather fires mid-layer (after block 4's store) and the hi one at
    layer end, so each collective overlaps the neighbouring layer's
    aggregation instead of serializing (SWDGE descriptor generation on the
    Q7s, ~3.4ns/row, is the phase floor; the collectives ride under it).
  - Aggregation (symmetric-normalized adjacency incl. self-loops) runs per
    128-destination-node block: fp8 source rows are fetched with dma_gather
    (512-idx calls, 32 descs/engine, single-packet) and summed on the PE as
    OUT_block += S_chunk.T @ MSG_chunk with S a host-built fp8 [128e, 128d]
    weight matrix. Rows are deduped per (source, dst-block): one gathered
    row serves all its edges into the block via multiple nonzeros in its
    S column (~10% fewer descriptors + bytes).
  - Layer 4 output G4 = H4 @ W4 is aggregated at fp8 256-wide (2 classes
    padded; 256B gather rows) and log_softmax is fused on-chip.
"""

import sys

sys.path.insert(0, "/opt/trn_rl_repo")

import numpy as np
import ml_dtypes

BF16 = ml_dtypes.bfloat16
F8 = ml_dtypes.float8_e4m3

# Problem constants (nn_GCN_39195871543847)
N, E, F_IN, HID, C = 10000, 160000, 2208, 512, 2
W_CORES = 8
RPC = N // W_CORES  # 1250 nodes per core
MB = 10  # 128-row blocks per core
RPAD = MB * 128  # 1280
SPLIT = 4  # blocks 0..SPLIT-1 -> lo half, SPLIT..9 -> hi half
RH = SPLIT * 128  # 512 rows in the lo half per core
RH_HI = (MB - SPLIT) * 128  # 768 rows in the hi half per core
GH = W_CORES * RH  # gathered lo height
GH_HI = W_CORES * RH_HI  # gathered hi height
KFC = (F_IN + 127) // 128  # 18 contraction chunks for layer 1
KFP = KFC * 128  # 2304
C_PAD = 256  # pad 2 output classes to 256 fp8 (256B gather rows)
CALL = 7  # 128-idx chunks per dma_gather call (896 idx = 56 descs/engine)
N_QUEUES = 4  # SWDGE queues for gather descriptor generation


def _install_drain_patch():
    """This container's walrus accepts at most one sync-wait per instruction;
    TileContext's final drain gets one wait per live semaphore. Split the
    extra waits onto single-wait NOPs."""
    import bass_rust
    import concourse.tile as tile
    from concourse.vector_clock import ScopedClock

    if getattr(tile.TileContext, "_drain_patch_installed", False):
        return

    def _drain_and_barrier(self, tick_clock, wait_clock):
        drain_inst = self.nc.sync.drain()
        wait_clock.add_sem_waits(
            drain_inst.ins, ScopedClock({None: tick_clock.global_clock})
        )
        si = drain_inst.ins.sync_info
        waits = list(si.on_wait or []) if si is not None else []
        if len(waits) > 1:
            si.on_wait = waits[:1]
            for w in waits[1:]:
                nop = self.nc.sync.nop(nofuse=True)
                nop.ins.sync_info = bass_rust.SyncInfo(on_wait=[w], on_update=[])
        self.nc.all_engine_barrier()
        assert self.sems is not None
        popped = self.nc._tile_sem_poison_stack.pop()
        assert popped is self._sem_poison
        self.nc.clear_and_free_semaphores(list(self.sems.allocated().values()))
        self.nc.all_engine_barrier()

    tile.TileContext._drain_and_barrier = _drain_and_barrier
    tile.TileContext._drain_patch_installed = True


# ----------------------------------------------------------------------------
# Host-side graph preprocessing
# ----------------------------------------------------------------------------


def _preprocess(edge_index):
    """Per core, per 128-dst block, per source class: dedup edges by source
    row, chunk the deduped slots, and build the S stack + gather indices.

    Source classes: 0 = local (source owned by this core; gathered from the
    core-local own_all copy with NO collective dependency — these fill the
    AllGather entry-latency dead zone at each layer boundary), 1 = remote lo
    half, 2 = remote hi half.

    Chunk stream order (per core, shared tb layout across cores):
      [loc b0..b9][lo b0..b9][hi b0..b9]
    """
    src = edge_index[0].astype(np.int64)
    dst = edge_index[1].astype(np.int64)
    loop = np.arange(N, dtype=np.int64)
    s = np.concatenate([src, loop])
    d = np.concatenate([dst, loop])
    deg = np.bincount(d, minlength=N).astype(np.float32)
    dinv = np.where(deg > 0, 1.0 / np.sqrt(deg), 0.0).astype(np.float32)
    norm = (dinv[s] * dinv[d]).astype(np.float64)

    core = d // RPC
    # per (core, block, class): deduped slot rows + per-edge (slot, mloc, w)
    slot_rows = {}
    edge_tuples = {}
    ka = np.zeros((MB, 3), np.int64)
    for c in range(W_CORES):
        m = core == c
        sc, dc, wc = s[m], d[m] - c * RPC, norm[m]
        s_core = sc // RPC
        s_loc = sc % RPC
        half = np.where(
            s_core == c, 0, np.where(s_loc < RH, 1, 2)
        ).astype(np.int64)
        g_row = np.where(
            s_core == c,
            s_loc,
            np.where(
                s_loc < RH, s_core * RH + s_loc, s_core * RH_HI + (s_loc - RH)
            ),
        )
        blk = dc // 128
        mloc = dc % 128
        for b in range(MB):
            for h in range(3):
                mm = (blk == b) & (half == h)
                rows = g_row[mm]
                ml = mloc[mm]
                ww = wc[mm]
                uniq, inv = np.unique(rows, return_inverse=True)
                slot_rows[(c, b, h)] = uniq
                edge_tuples[(c, b, h)] = (inv, ml, ww)
                ka[b, h] = max(ka[b, h], 1, (len(uniq) + 127) // 128)

    # stream layout: all loc runs, then all lo runs, then all hi runs
    seg_order = [(h, b) for h in range(3) for b in range(MB)]
    tb_off = {}
    t = 0
    for h, b in seg_order:
        tb_off[(b, h)] = t
        t += int(ka[b, h])
    T = t

    s_list, idx_list = [], []
    for c in range(W_CORES):
        S = np.zeros((T, 128, 128), np.float32)
        idx_flat = np.zeros(T * 128, np.int16)
        for h, b in seg_order:
            if (c, b, h) not in slot_rows:
                continue
            t0 = tb_off[(b, h)]
            uniq = slot_rows[(c, b, h)]
            inv, ml, ww = edge_tuples[(c, b, h)]
            k = np.arange(len(uniq))
            tt = t0 + k // 128
            kk = k % 128
            idx_flat[tt * 128 + kk] = uniq.astype(np.int16)
            # scatter-add per-edge weights into the slot's S column
            np.add.at(S, (tt[inv], kk[inv], ml), ww)
        lay16 = idx_flat.reshape(T * 8, 16).T  # [16, T*8]
        idx_list.append(np.tile(lay16, (8, 1)).astype(np.int16))
        # SBUF-resident layout [128 partitions(k), T, 128(m)]
        s_list.append(
            np.ascontiguousarray(S.transpose(1, 0, 2)).astype(F8)
        )
    ka_t = tuple(
        (int(ka[b, 0]), int(ka[b, 1]), int(ka[b, 2])) for b in range(MB)
    )
    return ka_t, s_list, idx_list


def _prep_inputs(x, edge_index, W1, b1, W2, b2, W3, b3, W4, b4):
    ka, s_list, idx_list = _preprocess(edge_index)

    # xT per core: [MB, 128(p), KFC, 128(j)]; xT[m,p,k,j] = x[c*RPC+m*128+j, k*128+p]
    xts = []
    for c in range(W_CORES):
        xp = np.zeros((RPAD, KFP), np.float32)
        xp[:RPC, :F_IN] = x[c * RPC : (c + 1) * RPC]
        xt = xp.reshape(MB, 128, KFC, 128).transpose(0, 3, 2, 1)
        xts.append(np.ascontiguousarray(xt).astype(BF16))

    W1p = np.zeros((KFP, HID), np.float32)
    W1p[:F_IN] = W1
    W1l = np.ascontiguousarray(
        W1p.reshape(KFC, 128, HID).transpose(1, 0, 2)
    ).astype(BF16)
    W2l = np.ascontiguousarray(W2.reshape(4, 128, HID).transpose(1, 0, 2)).astype(BF16)
    W3l = np.ascontiguousarray(W3.reshape(4, 128, HID).transpose(1, 0, 2)).astype(BF16)
    W4p = np.zeros((HID, C_PAD), np.float32)
    W4p[:, :C] = W4
    W4l = np.ascontiguousarray(
        W4p.reshape(4, 128, C_PAD).transpose(1, 0, 2)
    ).astype(BF16)

    b1r = np.broadcast_to(b1, (128, HID)).astype(np.float32).copy()
    b2r = np.broadcast_to(b2, (128, HID)).astype(np.float32).copy()
    b3r = np.broadcast_to(b3, (128, HID)).astype(np.float32).copy()
    b4r = np.zeros((128, C_PAD), np.float32)
    b4r[:, :C] = b4

    in_maps = []
    for c in range(W_CORES):
        in_maps.append(
            {
                "xT": xts[c],
                "W1l": W1l, "W2l": W2l, "W3l": W3l, "W4l": W4l,
                "b1r": b1r, "b2r": b2r, "b3r": b3r, "b4r": b4r,
                "S_in": s_list[c],
                "idx_in": idx_list[c],
            }
        )
    return ka, in_maps


# ----------------------------------------------------------------------------
# Bass kernel builder
# ----------------------------------------------------------------------------

_cache = {}


def _build(ka):
    import concourse.bass as bass
    import concourse.mybir as mybir
    from concourse.bacc import Bacc
    from concourse.tile import TileContext
    from concourse.masks import make_identity

    f32 = mybir.dt.float32
    bf16 = mybir.dt.bfloat16
    f8 = mybir.dt.float8e4
    i16 = mybir.dt.int16

    # chunk stream layout (must match _preprocess seg_order)
    seg_order = [(h, b) for h in range(3) for b in range(MB)]
    tb_off = {}
    t = 0
    for h, b in seg_order:
        tb_off[(b, h)] = t
        t += ka[b][h]
    T = t

    nc = Bacc(num_devices=W_CORES, num_swdge_queues=N_QUEUES)
    gq = [0]  # round-robin cursor over gather queues

    xT = nc.dram_tensor("xT", [MB, 128, KFC, 128], bf16, kind="ExternalInput")
    W1l = nc.dram_tensor("W1l", [128, KFC, HID], bf16, kind="ExternalInput")
    W2l = nc.dram_tensor("W2l", [128, 4, HID], bf16, kind="ExternalInput")
    W3l = nc.dram_tensor("W3l", [128, 4, HID], bf16, kind="ExternalInput")
    W4l = nc.dram_tensor("W4l", [128, 4, C_PAD], bf16, kind="ExternalInput")
    b1r = nc.dram_tensor("b1r", [128, HID], f32, kind="ExternalInput")
    b2r = nc.dram_tensor("b2r", [128, HID], f32, kind="ExternalInput")
    b3r = nc.dram_tensor("b3r", [128, HID], f32, kind="ExternalInput")
    b4r = nc.dram_tensor("b4r", [128, C_PAD], f32, kind="ExternalInput")
    S_in = nc.dram_tensor("S_in", [128, T, 128], f8, kind="ExternalInput")
    idx_in = nc.dram_tensor("idx_in", [128, T * 8], i16, kind="ExternalInput")
    out = nc.dram_tensor("out", [RPAD, C], f32, kind="ExternalOutput")

    # per-layer bounce halves + gathered halves (all fp8); own_all is a
    # core-local full copy serving the loc gather stream (no collective dep)
    l_wid = [HID, HID, HID, C_PAD]
    own_lo, own_hi, own_all, full_lo, full_hi = [], [], [], [], []
    for l in range(4):
        own_lo.append(nc.dram_tensor(f"own_lo{l}", [RH, l_wid[l]], f8, kind="Internal"))
        own_hi.append(
            nc.dram_tensor(f"own_hi{l}", [RH_HI, l_wid[l]], f8, kind="Internal")
        )
        own_all.append(
            nc.dram_tensor(f"own_all{l}", [RPAD, l_wid[l]], f8, kind="Internal")
        )
        full_lo.append(
            nc.dram_tensor(f"full_lo{l}", [GH, l_wid[l]], f8, kind="Internal",
                           addr_space="Shared")
        )
        full_hi.append(
            nc.dram_tensor(f"full_hi{l}", [GH_HI, l_wid[l]], f8, kind="Internal",
                           addr_space="Shared")
        )

    rg = [list(range(W_CORES))]

    with TileContext(nc) as tc:
        with (
            tc.tile_pool(name="const", bufs=1) as cpool,
            tc.tile_pool(name="work", bufs=2) as wpool,
            tc.tile_pool(name="psum", bufs=2, space="PSUM") as ppool,
        ):
            relu = mybir.ActivationFunctionType.Relu

            # class chunk ranges (chunks are contiguous across blocks within
            # a class in the h-major stream layout) + index-count registers
            cls_rng = {}
            for h in range(3):
                cls_rng[h] = (
                    tb_off[(0, h)], tb_off[(MB - 1, h)] + ka[MB - 1][h]
                )
            sizes = {CALL * 128}
            for h in range(3):
                n = cls_rng[h][1] - cls_rng[h][0]
                if n % CALL:
                    sizes.add((n % CALL) * 128)
            r_cnt = {n: nc.gpsimd.to_reg(n) for n in sizes}

            # ---- layer-1 GEMM inputs first (critical path) -------------------
            # Load W1 in 3 chunk groups so the first matmul (which reads only
            # chunk 0) isn't gated on the full 2.4MB transfer.
            W1_sb = []
            for k0 in range(0, KFC, 6):
                k1 = min(k0 + 6, KFC)
                wg = cpool.tile([128, k1 - k0, HID], bf16, tag=f"w1g{k0}")
                nc.sync.dma_start(out=wg[:], in_=W1l[:, k0:k1, :])
                W1_sb.append(wg)

            def allgather(own, full):
                nc.gpsimd.collective_compute(
                    "AllGather",
                    mybir.AluOpType.bypass,
                    ins=[own[:]],
                    outs=[full[:]],
                    replica_groups=rg,
                )

            def store_own(lslot, b, tile):
                if b < SPLIT:
                    nc.sync.dma_start(
                        out=own_lo[lslot][b * 128 : (b + 1) * 128, :], in_=tile[:]
                    )
                else:
                    r0 = (b - SPLIT) * 128
                    nc.sync.dma_start(
                        out=own_hi[lslot][r0 : r0 + 128, :], in_=tile[:]
                    )
                nc.sync.dma_start(
                    out=own_all[lslot][b * 128 : (b + 1) * 128, :], in_=tile[:]
                )

            def gemm_l1():
                for m in range(MB):
                    xm = wpool.tile([128, KFC, 128], bf16, tag="xm", bufs=3)
                    nc.sync.dma_start(out=xm[:], in_=xT[m])
                    ps = ppool.tile([128, HID], f32, tag="gps")
                    for k in range(KFC):
                        nc.tensor.matmul(
                            ps[:],
                            lhsT=xm[:, k, :],
                            rhs=W1_sb[k // 6][:, k % 6, :],
                            start=(k == 0),
                            stop=(k == KFC - 1),
                        )
                    gb = wpool.tile([128, HID], f8, tag="gb", bufs=3)
                    nc.scalar.copy(gb[:], ps[:])
                    store_own(0, m, gb)
                    if m == SPLIT - 1:
                        allgather(own_lo[0], full_lo[0])
                    elif m == MB - 1:
                        allgather(own_hi[0], full_hi[0])

            gemm_l1()

            # ---- remaining resident tensors (overlap the first collective) ---
            # idx first: the layer-0 loc desc-gen needs it long before the
            # first aggregation matmul needs S.
            idx_sb = cpool.tile([128, T * 8], i16)
            nc.sync.dma_start(out=idx_sb[:], in_=idx_in[:])
            S_sb = cpool.tile([128, T, 128], f8)
            nc.sync.dma_start(out=S_sb[:], in_=S_in[:])
            W2_sb = cpool.tile([128, 4, HID], bf16)
            nc.sync.dma_start(out=W2_sb[:], in_=W2l[:])
            W3_sb = cpool.tile([128, 4, HID], bf16)
            nc.sync.dma_start(out=W3_sb[:], in_=W3l[:])
            W4_sb = cpool.tile([128, 4, C_PAD], bf16)
            nc.sync.dma_start(out=W4_sb[:], in_=W4l[:])
            b_sb = []
            for nm, srcb in (("b1", b1r), ("b2", b2r), ("b3", b3r)):
                tle = cpool.tile([128, HID], f32, tag=f"bias_{nm}")
                nc.sync.dma_start(out=tle[:], in_=srcb[:])
                b_sb.append(tle)
            b4_sb = cpool.tile([128, C_PAD], f32)
            nc.sync.dma_start(out=b4_sb[:], in_=b4r[:])
            id_bf = cpool.tile([128, 128], bf16)
            make_identity(nc, id_bf[:])

            def issue_class_calls(h, fsrc, w, tiles):
                """Issue the gather calls for class h's whole chunk run
                (windows span block boundaries — one call stream per class);
                record tiles keyed by window-start chunk index."""
                t0, t1 = cls_rng[h]
                for tw in range(t0, t1, CALL):
                    nk = min(CALL, t1 - tw)
                    msg = wpool.tile([128, CALL, w], f8, tag=f"msg{w}", bufs=10)
                    nc.gpsimd.dma_gather(
                        out_ap=msg[:, :nk, :],
                        in_ap=fsrc[:],
                        idxs_ap=idx_sb[:, tw * 8 : (tw + nk) * 8],
                        num_idxs=nk * 128,
                        num_idxs_reg=r_cnt[nk * 128],
                        elem_size=w,
                        queue_num=gq[0],
                    )
                    gq[0] = (gq[0] + 1) % N_QUEUES
                    tiles[tw] = msg

            def half_mms(ps, b, hs, tiles):
                """Accumulate block b's chunks for the classes in hs into ps."""
                runs = []
                for h in hs:
                    c0 = cls_rng[h][0]
                    t0 = tb_off[(b, h)]
                    for t in range(t0, t0 + ka[b][h]):
                        runs.append((t, c0 + ((t - c0) // CALL) * CALL))
                for i, (t, tw) in enumerate(runs):
                    msg = tiles[tw]
                    nc.tensor.matmul(
                        ps[:],
                        lhsT=S_sb[:, t, :],
                        rhs=msg[:, t - tw, :],
                        start=(i == 0),
                        stop=(i == len(runs) - 1),
                    )

            def logsoftmax_block(ps, hacc, m):
                lg0 = wpool.tile([128, C_PAD], f32, tag="lg0")
                nc.vector.tensor_add(out=lg0[:], in0=ps[:], in1=hacc[:])
                lg = wpool.tile([128, C_PAD], f32, tag="lg")
                nc.vector.tensor_add(out=lg[:], in0=lg0[:], in1=b4_sb[:])
                mx = wpool.tile([128, 1], f32, tag="mx")
                nc.vector.tensor_reduce(
                    out=mx[:], in_=lg[:, :C], axis=mybir.AxisListType.X,
                    op=mybir.AluOpType.max,
                )
                t2 = wpool.tile([128, C], f32, tag="t2")
                nc.vector.tensor_scalar(
                    out=t2[:], in0=lg[:, :C], scalar1=mx[:], scalar2=None,
                    op0=mybir.AluOpType.subtract,
                )
                e2 = wpool.tile([128, C], f32, tag="e2")
                nc.scalar.activation(e2[:], t2[:], mybir.ActivationFunctionType.Exp)
                sm = wpool.tile([128, 1], f32, tag="sm")
                nc.vector.tensor_reduce(
                    out=sm[:], in_=e2[:], axis=mybir.AxisListType.X,
                    op=mybir.AluOpType.add,
                )
                ls = wpool.tile([128, 1], f32, tag="ls")
                nc.scalar.activation(ls[:], sm[:], mybir.ActivationFunctionType.Ln)
                o2 = wpool.tile([128, C], f32, tag="o2")
                nc.vector.tensor_scalar(
                    out=o2[:], in0=t2[:], scalar1=ls[:], scalar2=None,
                    op0=mybir.AluOpType.subtract,
                )
                nc.sync.dma_start(out=out[m * 128 : (m + 1) * 128, :], in_=o2[:])

            def block_tail(l, b, ps, hacc, bias_t, mode):
                """Bias + lo-half partial + relu + transpose + next-layer GEMM
                + store for one finished block (or the final classifier)."""
                if mode[0] == "final":
                    logsoftmax_block(ps, hacc, b)
                    return
                h0 = wpool.tile([128, HID], f32, tag="h0", bufs=3)
                nc.vector.tensor_add(out=h0[:], in0=ps[:], in1=hacc[:])
                hf = wpool.tile([128, HID], f32, tag="hf", bufs=3)
                nc.vector.tensor_add(out=hf[:], in0=h0[:], in1=bias_t[:])
                hb = wpool.tile([128, HID], bf16, tag="hb", bufs=3)
                nc.scalar.activation(hb[:], hf[:], relu)
                ht = wpool.tile([128, 4, 128], bf16, tag="ht", bufs=4)
                for gg in range(4):
                    tp = ppool.tile([128, 128], bf16, tag="tps", bufs=1)
                    nc.tensor.transpose(
                        tp[:], hb[:, gg * 128 : (gg + 1) * 128], id_bf[:]
                    )
                    nc.vector.tensor_copy(out=ht[:, gg, :], in_=tp[:])
                _, w_sb, lnext = mode
                wid = l_wid[lnext]
                gp = ppool.tile([128, wid], f32, tag="gps")
                for k in range(4):
                    nc.tensor.matmul(
                        gp[:],
                        lhsT=ht[:, k, :],
                        rhs=w_sb[:, k, :],
                        start=(k == 0),
                        stop=(k == 3),
                    )
                gb = wpool.tile([128, wid], f8, tag="gb", bufs=3)
                nc.scalar.copy(gb[:], gp[:])
                store_own(lnext, b, gb)

            def layer(l, bias_t, mode):
                """One fused layer in two phases: phase A accumulates every
                block's lo-half chunks into PSUM and spills to SBUF (only
                needs full_lo[l], which was gathered by mid-layer l-1);
                phase B adds the hi-half (whose AllGather completes early in
                this layer) and runs the per-block tail. The next layer's lo
                AllGather fires from block SPLIT-1's store, the hi one at
                layer end."""
                w = l_wid[l]
                pw = HID if mode[0] != "final" else C_PAD
                tiles = {}
                # loc calls first: they depend only on this core's own stores
                # from the previous layer, so their desc-gen + MMs fill the
                # AllGather entry-latency window at the layer boundary.
                issue_class_calls(0, own_all[l], w, tiles)
                issue_class_calls(1, full_lo[l], w, tiles)
                haccs = {}
                for b in range(MB):
                    psA = ppool.tile([128, pw], f32, tag="apsA", bufs=2)
                    half_mms(psA, b, (0, 1), tiles)
                    hc = wpool.tile([128, pw], f32, tag="hacc", bufs=10)
                    nc.scalar.copy(hc[:], psA[:])
                    haccs[b] = hc
                issue_class_calls(2, full_hi[l], w, tiles)
                for b in range(MB):
                    psB = ppool.tile([128, pw], f32, tag="apsB", bufs=2)
                    half_mms(psB, b, (2,), tiles)
                    block_tail(l, b, psB, haccs[b], bias_t, mode)
                    if mode[0] != "final" and b == SPLIT - 1:
                        allgather(own_lo[mode[2]], full_lo[mode[2]])
                if mode[0] != "final":
                    allgather(own_hi[mode[2]], full_hi[mode[2]])

            # ---- layers ----------------------------------------------------
            layer(0, b_sb[0], ("gemm", W2_sb, 1))
            layer(1, b_sb[1], ("gemm", W3_sb, 2))
            layer(2, b_sb[2], ("gemm", W4_sb, 3))
            layer(3, None, ("final",))

    nc.compile()
    return nc


# ----------------------------------------------------------------------------
# Entry point
# ----------------------------------------------------------------------------


def kernel(x, edge_index, batch, W1, b1, W2, b2, W3, b3, W4, b4, _trace=False):
    _install_drain_patch()
    from concourse.bass_utils import run_bass_kernel_spmd

    ka, in_maps = _prep_inputs(
        np.asarray(x, np.float32),
        np.asarray(edge_index),
        np.asarray(W1, np.float32), np.asarray(b1, np.float32),
        np.asarray(W2, np.float32), np.asarray(b2, np.float32),
        np.asarray(W3, np.float32), np.asarray(b3, np.float32),
        np.asarray(W4, np.float32), np.asarray(b4, np.float32),
    )
    key = tuple(ka)
    if key not in _cache:
        _cache[key] = _build(ka)
    nc = _cache[key]
    res = run_bass_kernel_spmd(
        nc, in_maps, core_ids=list(range(W_CORES)), trace=_trace
    )
    outp = np.concatenate(
        [res.results[c]["out"][:RPC] for c in range(W_CORES)], axis=0
    ).astype(np.float32)
    if _trace:
        return outp, res
    return outp
